# revision 29
# baseline (speedup 1.0000x reference)
"""Single-head causal attention (B=8, T=2048, C=1024, H=128) on 8 TRN2 NeuronCores.

Sharding: data-parallel over batch B — core b computes attention for x[b].
Host-side prep per core: x[b] is transposed to xT [C, T] (contraction dim C on
SBUF partitions) and the softmax scale C**-0.5 is folded into Wq. The kernel
computes in the transposed orientation; the host untransposes the [H, T] output.

Device kernel per core (ST-direct, all bf16 matmuls):
  warmup: junk matmuls ramp the PE p-state while the x DMA lands.
  projections per 512-col n-chunk; chunk 0 interleaves Q/K/V per c-chunk so
  the PE chases the x DMA arrival order. Q,K accumulate in one [128,1024]
  PSUM pair tile (chunks 0,1) or two single banks (chunks 2,3 — those
  interleave with attention, which owns the pair pool); one ACT copy per
  chunk moves Q,K to SBUF, DVE moves V, and a DMA-xbar transpose yields
  V [T,H] tiles per chunk.
  attention, software pipeline over (g, pair), scores one pair ahead of AV,
  with projection chunks 2,3 slotted between the first pairs:
    ST_jg = KT_j.T @ QT_g   [128 s, <=512 q] x2 per pair  (PE)
    diagonal tiles are column-trimmed to 512/384/256/128 (causally dead
    columns never computed), masked post-exp via affine_select (GpSimd)
    expST = exp(ST)         -> SBUF bf16                  (ACT)
    row-sums: GpSimd folds e0+e1, DVE accumulates into f32 (not PE)
    outT_g += V_j.T @ expST [H, 512] PSUM                 (PE)
  per g epilogue (own unit, placed 2+ units later so the PE never waits on
  DVE): ones-matmul broadcasts the partition row-sum, DVE reciprocal +
  multiply, DMA out on the sync queue. The last pair of the last q-block
  takes a short-latency path: additive -30000 mask pre-exp (DVE), row-sums
  finished by PE ones-matmuls on the exp tile, halved rec/mul/DMA epilogue
  on two queues.
"""

import os
from contextlib import ExitStack

import numpy as np
import ml_dtypes

B, T, C, H = 8, 2048, 1024, 128
P = 128
NT = T // P  # 16 s-tiles per core
NCC = C // P  # 8 contraction chunks
QB = 512  # q-block width
NQB = T // QB  # 4 q-blocks
N_CORES = 8
SCALE = float(C) ** -0.5

_CACHE = {}


def _build():
    import concourse.bass as bass  # noqa: F401
    import concourse.mybir as mybir
    import concourse.tile as tile
    from concourse import bacc

    dt = mybir.dt
    bf16 = dt.bfloat16
    f32 = dt.float32

    nc = bacc.Bacc("TRN2", target_bir_lowering=False, debug=False)
    xT = nc.dram_tensor("xT", [C, T], bf16, kind="ExternalInput").ap()
    wq = nc.dram_tensor("wq", [P, NCC * H], bf16, kind="ExternalInput").ap()
    wk = nc.dram_tensor("wk", [P, NCC * H], bf16, kind="ExternalInput").ap()
    wv = nc.dram_tensor("wv", [P, NCC * H], bf16, kind="ExternalInput").ap()
    outT = nc.dram_tensor("outT", [H, T], f32, kind="ExternalOutput").ap()

    with tile.TileContext(nc) as tc, ExitStack() as ctx:
        # --- input DMA: wq leads scalar while x c0 leads sync so the first
        # matmul's two dependencies arrive in parallel; wv on gpsimd ---
        wpool = ctx.enter_context(tc.tile_pool(name="wpool", bufs=1))
        xpool = ctx.enter_context(tc.tile_pool(name="xpool", bufs=1))
        w_sb = {
            name: wpool.tile([P, NCC * H], bf16, name=f"{name}_sb")
            for name in ("wq", "wk", "wv")
        }
        xt_sb = xpool.tile([P, NCC * T], bf16)

        def xdma(eng, n2, c):
            eng.dma_start(
                xt_sb[:, c * T + n2 * 1024 : c * T + (n2 + 1) * 1024],
                xT[c * P : (c + 1) * P, n2 * 1024 : (n2 + 1) * 1024],
            )

        nc.scalar.dma_start(w_sb["wq"], wq)
        xdma(nc.sync, 0, 0)
        nc.sync.dma_start(w_sb["wk"], wk)
        nc.gpsimd.dma_start(w_sb["wv"], wv)
        for n2 in range(2):
            for c in range(NCC):
                if n2 == 0 and c == 0:
                    continue
                xdma(nc.sync if (c + n2) % 2 == 0 else nc.scalar, n2, c)

        consts = ctx.enter_context(tc.tile_pool(name="consts", bufs=1))
        ones512 = consts.tile([P, QB], bf16)
        nc.vector.memset(ones512, 1.0)
        # additive causal mask for the last pair: slice [384:384+w] fills
        # -30000 where local col f < partition s_l
        MW = QB + 384
        mbig = consts.tile([P, MW], f32)
        nc.gpsimd.memset(mbig, 0.0)
        nc.gpsimd.affine_select(
            out=mbig,
            in_=mbig,
            compare_op=mybir.AluOpType.is_ge,
            fill=-30000.0,
            base=-384,
            pattern=[[1, MW]],
            channel_multiplier=-1,
        )
        prime = consts.tile([P, P], f32)
        nc.vector.tensor_copy(prime, mbig[:, :P])

        qkv = ctx.enter_context(tc.tile_pool(name="qkv", bufs=1))
        qkt_sb = qkv.tile([P, 2 * T], bf16)  # [:, :T] = QT, [:, T:] = KT
        qt_sb = qkt_sb[:, :T]
        kt_sb = qkt_sb[:, T:]
        vt_sb = qkv.tile([P, T], bf16)
        v_sb = qkv.tile([P, NT * H], bf16)
        v3 = v_sb.rearrange("p (t h) -> p t h", t=NT)

        # PSUM: att pairs 2x[128,1024] = 4 banks, pav 2, small 2
        ps_pair = ctx.enter_context(tc.tile_pool(name="ps_pair", bufs=2, space="PSUM"))
        ps_av = ctx.enter_context(tc.tile_pool(name="ps_av", bufs=2, space="PSUM"))
        ps_small = ctx.enter_context(
            tc.tile_pool(name="ps_small", bufs=2, space="PSUM")
        )

        # --- PE warmup: ramp p-state while DMA lands (results discarded) ---
        for i in range(9):
            pv = ps_small.tile([P, QB], f32, name=f"warm{i}", tag="small")
            nc.tensor.matmul(pv, ones512[:, :P], ones512, start=True, stop=True)

        def emit_proj_chunk(n, pairqk):
            xs = lambda c: xt_sb[:, c * T + n * QB : c * T + (n + 1) * QB]
            if pairqk:
                pt = ps_pair.tile([P, 1024], f32, name=f"pqk{n}", tag="pair")
                pv = ps_small.tile([P, QB], f32, name=f"pv{n}", tag="small")
                accs = {"wq": pt[:, :QB], "wk": pt[:, QB:], "wv": pv}
                loop = (
                    [(w, c) for c in range(NCC) for w in ("wq", "wk", "wv")]
                    if n == 0
                    else [(w, c) for w in ("wq", "wk", "wv") for c in range(NCC)]
                )
                for wname, c in loop:
                    nc.tensor.matmul(
                        accs[wname],
                        w_sb[wname][:, c * H : (c + 1) * H],
                        xs(c),
                        start=(c == 0),
                        stop=(c == NCC - 1),
                    )
                dst = qkt_sb.rearrange("p (two t) -> p two t", two=2)[
                    :, :, n * QB : (n + 1) * QB
                ]
                nc.scalar.activation(
                    dst,
                    pt.rearrange("p (two q) -> p two q", two=2),
                    mybir.ActivationFunctionType.Copy,
                )
                nc.vector.tensor_copy(vt_sb[:, n * QB : (n + 1) * QB], pv)
            nc.sync.dma_start(
                v3[:, n * 4 : (n + 1) * 4, :],
                vt_sb[:, n * QB : (n + 1) * QB],
                transpose=True,
            )

        def emit_proj_one(n, wname):
            """Single-projection unit for chunks interleaved with attention."""
            dst = {"wq": qt_sb, "wk": kt_sb, "wv": vt_sb}[wname]
            acc = ps_small.tile([P, QB], f32, name=f"p{wname}{n}", tag="small")
            for c in range(NCC):
                nc.tensor.matmul(
                    acc,
                    w_sb[wname][:, c * H : (c + 1) * H],
                    xt_sb[:, c * T + n * QB : c * T + (n + 1) * QB],
                    start=(c == 0),
                    stop=(c == NCC - 1),
                )
            if wname == "wv":
                nc.vector.tensor_copy(dst[:, n * QB : (n + 1) * QB], acc)
                nc.sync.dma_start(
                    v3[:, n * 4 : (n + 1) * 4, :],
                    vt_sb[:, n * QB : (n + 1) * QB],
                    transpose=True,
                )
            else:
                nc.scalar.activation(
                    dst[:, n * QB : (n + 1) * QB],
                    acc,
                    mybir.ActivationFunctionType.Copy,
                )

        # --- attention ---
        expst_pool = ctx.enter_context(tc.tile_pool(name="expst_pool", bufs=6))
        sacc_pool = ctx.enter_context(tc.tile_pool(name="sacc_pool", bufs=2))
        tp_pool = ctx.enter_context(tc.tile_pool(name="tp_pool", bufs=2))
        sbf_pool = ctx.enter_context(tc.tile_pool(name="sbf_pool", bufs=2))
        rec_pool = ctx.enter_context(tc.tile_pool(name="rec_pool", bufs=2))
        outp = ctx.enter_context(tc.tile_pool(name="outp", bufs=2))

        state = {}  # per-g: pav, sacc, then sbf
        GL = NQB - 1
        LAST = (GL, 2 * GL + 1)

        def tile_geom(g, p, k):
            """-> (d, col0, width, psum offset) for slice k of pair (g,p)."""
            j = 2 * p + k
            d = j - 4 * g
            if 0 <= d <= 3:
                w = QB - 128 * d
                # keep start=True slices in separate PSUM banks
                off = 0 if k == 0 else QB
                return d, 128 * d, w, off
            return None, 0, QB, k * QB

        def emit_scores(g, p):
            qs0 = g * QB
            ps = ps_pair.tile([P, 1024], f32, name=f"ps_{g}_{p}", tag="pair")
            geom = [tile_geom(g, p, k) for k in range(2)]
            for k in range(2):
                j = 2 * p + k
                d, col0, w, off = geom[k]
                nc.tensor.matmul(
                    ps[:, off : off + w],
                    kt_sb[:, j * P : (j + 1) * P],
                    qt_sb[:, qs0 + col0 : qs0 + QB],
                    start=True,
                    stop=True,
                )
            if (g, p) == LAST:
                for k in range(2):
                    d, col0, w, off = geom[k]
                    nc.vector.tensor_add(
                        ps[:, off : off + w],
                        ps[:, off : off + w],
                        mbig[:, 384 : 384 + w],
                    )
            w0 = geom[0][2]
            w1 = geom[1][2]
            expst = expst_pool.tile([P, 1024], bf16, name=f"e{g}_{p}", tag="expst")
            if geom[1][3] == w0:  # contiguous psum: single exp
                nc.scalar.activation(
                    expst[:, : w0 + w1],
                    ps[:, : w0 + w1],
                    mybir.ActivationFunctionType.Exp,
                )
            else:
                nc.scalar.activation(
                    expst[:, :w0], ps[:, :w0], mybir.ActivationFunctionType.Exp
                )
                nc.scalar.activation(
                    expst[:, w0 : w0 + w1],
                    ps[:, QB : QB + w1],
                    mybir.ActivationFunctionType.Exp,
                )
            if (g, p) != LAST:
                for k in range(2):
                    d, col0, w, off = geom[k]
                    if d is not None:
                        e0 = 0 if k == 0 else w0
                        nc.gpsimd.affine_select(
                            out=expst[:, e0 : e0 + w],
                            in_=expst[:, e0 : e0 + w],
                            compare_op=mybir.AluOpType.is_ge,
                            fill=0.0,
                            base=0,
                            pattern=[[1, w]],
                            channel_multiplier=-1,
                        )
            return expst

        def emit_av(g, p, expst):
            npair = 2 * g + 2
            njt = 4 * g + 4
            if p == 0:
                state[g] = [
                    ps_av.tile([P, QB], f32, name=f"pav{g}", tag="pav"),
                    sacc_pool.tile([P, QB], f32, name=f"sacc{g}", tag="sacc"),
                    None,
                ]
            pav, sacc = state[g][0], state[g][1]
            geom = [tile_geom(g, p, k) for k in range(2)]
            w0 = geom[0][2]
            eslice = [expst[:, :w0], expst[:, w0 : w0 + geom[1][2]]]
            for k in range(2):
                j = 2 * p + k
                d, col0, w, off = geom[k]
                nc.tensor.matmul(
                    pav[:, col0:QB],
                    v_sb[:, j * H : (j + 1) * H],
                    eslice[k],
                    start=(j == 0),
                    stop=(j == njt - 1),
                )
            if g == GL and p == npair - 2:
                # row-sums for the last two pairs finish on PE: partial
                # ones-matmul over the DVE-accumulated pairs 0..npair-3, then
                # this pair's two slices
                psum = ps_small.tile([P, QB], f32, name=f"psum{g}", tag="small")
                state[g].append(psum)
                nc.tensor.matmul(
                    psum, ones512[:, :P], state[g][2], start=True, stop=False
                )
                for k in range(2):
                    d, col0, w, off = geom[k]
                    nc.tensor.matmul(
                        psum[:, col0:QB],
                        ones512[:, :P],
                        eslice[k],
                        start=False,
                        stop=False,
                    )
                return
            if (g, p) == LAST:
                psum = state[g][3]
                for k in range(2):
                    d, col0, w, off = geom[k]
                    nc.tensor.matmul(
                        psum[:, col0:QB],
                        ones512[:, :P],
                        eslice[k],
                        start=False,
                        stop=(k == 1),
                    )
                qs0 = g * QB
                for h in range(2):
                    hs = slice(h * 256, (h + 1) * 256)
                    rec = rec_pool.tile([P, QB], f32, name=f"rec{g}{h}", tag="rec")
                    nc.vector.reciprocal_approx_fast(rec[:, :256], psum[:, hs])
                    o = outp.tile([P, QB], f32, name=f"o{g}{h}", tag="o")
                    nc.vector.tensor_mul(o[:, :256], pav[:, hs], rec[:, :256])
                    eng = nc.sync if h == 0 else nc.scalar
                    eng.dma_start(
                        outT[:, qs0 + h * 256 : qs0 + (h + 1) * 256], o[:, :256]
                    )
                return
            # row-sum accumulation: GpSimd folds, DVE accumulates
            if geom[0][0] is None and geom[1][0] is None:
                if p == 0:
                    nc.vector.tensor_add(sacc, eslice[0], eslice[1])
                else:
                    tp = tp_pool.tile([P, QB], bf16, name=f"tp{g}_{p}", tag="tp")
                    nc.vector.tensor_add(tp, eslice[0], eslice[1])
                    nc.vector.tensor_add(sacc, sacc, tp)
            else:
                for k in range(2):
                    d, col0, w, off = geom[k]
                    if p == 0 and k == 0:
                        nc.vector.tensor_copy(sacc[:, col0:QB], eslice[k])
                    else:
                        nc.vector.tensor_add(
                            sacc[:, col0:QB], sacc[:, col0:QB], eslice[k]
                        )
            if (g != GL and p == npair - 1) or (g == GL and p == npair - 3):
                sbf = sbf_pool.tile([P, QB], bf16, name=f"sbf{g}", tag="sbf")
                nc.vector.tensor_copy(sbf, sacc)
                state[g][2] = sbf

        def emit_ep(g):
            qs0 = g * QB
            psum = ps_small.tile([P, QB], f32, name=f"psum{g}", tag="small")
            nc.tensor.matmul(psum, ones512[:, :P], state[g][2], start=True, stop=True)
            rec = rec_pool.tile([P, QB], f32, name=f"rec{g}", tag="rec")
            nc.vector.reciprocal_approx_fast(rec, psum)
            o = outp.tile([P, QB], f32, name=f"o{g}", tag="o")
            nc.vector.tensor_mul(o, state[g][0], rec)
            nc.sync.dma_start(outT[:, qs0 : qs0 + QB], o)

        # --- emission order: proj chunks 2,3 split per-projection and
        # slotted between attention pairs so the PE hides exp latency ---
        S, AV, PC, P1, EP = "S", "AV", "P", "p", "E"
        order = [
            (PC, 0), (PC, 1), (S, 0, 0), (S, 0, 1), (P1, 2, "wq"), (AV, 0, 0),
            (P1, 2, "wk"), (AV, 0, 1), (P1, 2, "wv"), (S, 1, 0), (S, 1, 1),
            (P1, 3, "wq"), (AV, 1, 0), (P1, 3, "wk"), (EP, 0), (AV, 1, 1),
            (S, 1, 2), (P1, 3, "wv"), (S, 1, 3), (AV, 1, 2), (S, 2, 0),
            (AV, 1, 3), (S, 2, 1), (AV, 2, 0), (S, 2, 2), (EP, 1), (AV, 2, 1),
            (S, 2, 3), (AV, 2, 2), (S, 2, 4), (AV, 2, 3), (S, 2, 5),
            (AV, 2, 4), (S, 3, 0), (AV, 2, 5), (S, 3, 1), (AV, 3, 0),
            (S, 3, 2), (EP, 2), (AV, 3, 1), (S, 3, 3), (AV, 3, 2), (S, 3, 4),
            (AV, 3, 3), (S, 3, 5), (AV, 3, 4), (S, 3, 6), (AV, 3, 5),
            (S, 3, 7), (AV, 3, 6), (AV, 3, 7),
        ]

        exps = {}
        for u in order:
            if u[0] == PC:
                emit_proj_chunk(u[1], pairqk=True)
            elif u[0] == P1:
                emit_proj_one(u[1], u[2])
            elif u[0] == S:
                exps[(u[1], u[2])] = emit_scores(u[1], u[2])
            elif u[0] == EP:
                emit_ep(u[1])
            else:
                emit_av(u[1], u[2], exps.pop((u[1], u[2])))

    nc.compile()
    return nc


def _get_bass():
    if "k" not in _CACHE:
        _CACHE["k"] = _build()
    return _CACHE["k"]


LAST_RESULT = None  # BassKernelResults of the most recent kernel() call


def kernel(x, Wq, Wk, Wv):
    global LAST_RESULT
    from concourse.bass_utils import run_bass_kernel_spmd

    np_dt = ml_dtypes.bfloat16

    def _wlayout(w):  # [C, H] -> [P, NCC*H]: sbuf layout, contiguous DMA
        w = np.asarray(w, np.float32).reshape(NCC, P, H).transpose(1, 0, 2)
        return np.ascontiguousarray(w.reshape(P, NCC * H)).astype(np_dt)

    wq_s = _wlayout(np.asarray(Wq, np.float32) * SCALE)
    wk_s = _wlayout(Wk)
    wv_s = _wlayout(Wv)
    x = np.asarray(x, np.float32)

    in_maps = []
    for b in range(N_CORES):
        in_maps.append(
            {
                "xT": np.ascontiguousarray(x[b].T).astype(np_dt),
                "wq": wq_s,
                "wk": wk_s,
                "wv": wv_s,
            }
        )

    nc = _get_bass()
    res = run_bass_kernel_spmd(nc, in_maps, core_ids=list(range(N_CORES)))
    LAST_RESULT = res
    return np.stack(
        [np.ascontiguousarray(r["outT"].T) for r in res.results], axis=0
    )


# revision 30
# speedup vs baseline: 1.1120x; 1.1120x over previous
"""Single-head causal attention (B=8, T=2048, C=1024, H=128) on 8 TRN2 NeuronCores.

Sharding: data-parallel over batch B — core b computes attention for x[b].
Host-side prep per core: x[b] is transposed to xT [C, T] (contraction dim C on
SBUF partitions) and the softmax scale C**-0.5 is folded into Wq. The kernel
computes in the transposed orientation; the host untransposes the [H, T] output.

Device kernel per core (ST-direct, all bf16 matmuls):
  warmup: junk matmuls ramp the PE p-state while the x DMA lands.
  projections per 512-col n-chunk; chunk 0 interleaves Q/K/V per c-chunk so
  the PE chases the x DMA arrival order. Q,K accumulate in one [128,1024]
  PSUM pair tile (chunks 0,1) or two single banks (chunks 2,3 — those
  interleave with attention, which owns the pair pool); one ACT copy per
  chunk moves Q,K to SBUF, DVE moves V, and a DMA-xbar transpose yields
  V [T,H] tiles per chunk.
  attention, software pipeline over (g, pair), scores one pair ahead of AV,
  with projection chunks 2,3 slotted between the first pairs:
    ST_jg = KT_j.T @ QT_g   [128 s, <=512 q] x2 per pair  (PE)
    diagonal tiles are column-trimmed to 512/384/256/128 (causally dead
    columns never computed), masked post-exp via affine_select (GpSimd)
    expST = exp(ST)         -> SBUF bf16                  (ACT)
    row-sums: GpSimd folds e0+e1, DVE accumulates into f32 (not PE)
    outT_g += V_j.T @ expST [H, 512] PSUM                 (PE)
  per g epilogue (own unit, placed 2+ units later so the PE never waits on
  DVE): ones-matmul broadcasts the partition row-sum, DVE reciprocal +
  multiply, DMA out on the sync queue. The last pair of the last q-block
  takes a short-latency path: additive -30000 mask pre-exp (DVE), row-sums
  finished by PE ones-matmuls on the exp tile, halved rec/mul/DMA epilogue
  on two queues.
"""

import os
from contextlib import ExitStack

import numpy as np
import ml_dtypes

B, T, C, H = 8, 2048, 1024, 128
P = 128
NT = T // P  # 16 s-tiles per core
NCC = C // P  # 8 contraction chunks
QB = 512  # q-block width
NQB = T // QB  # 4 q-blocks
N_CORES = 8
SCALE = float(C) ** -0.5

_CACHE = {}


def _build():
    import concourse.bass as bass  # noqa: F401
    import concourse.mybir as mybir
    import concourse.tile as tile
    from concourse import bacc

    dt = mybir.dt
    bf16 = dt.bfloat16
    f32 = dt.float32

    nc = bacc.Bacc("TRN2", target_bir_lowering=False, debug=False)
    xT = nc.dram_tensor("xT", [C, T], bf16, kind="ExternalInput").ap()
    wq = nc.dram_tensor("wq", [P, NCC * H], bf16, kind="ExternalInput").ap()
    wk = nc.dram_tensor("wk", [P, NCC * H], bf16, kind="ExternalInput").ap()
    wv = nc.dram_tensor("wv", [P, NCC * H], bf16, kind="ExternalInput").ap()
    outT = nc.dram_tensor("outT", [H, T], f32, kind="ExternalOutput").ap()

    with tile.TileContext(nc) as tc, ExitStack() as ctx:
        # --- input DMA: wq leads scalar while x c0 leads sync so the first
        # matmul's two dependencies arrive in parallel; wv on gpsimd ---
        wpool = ctx.enter_context(tc.tile_pool(name="wpool", bufs=1))
        xpool = ctx.enter_context(tc.tile_pool(name="xpool", bufs=1))
        w_sb = {
            name: wpool.tile([P, NCC * H], bf16, name=f"{name}_sb")
            for name in ("wq", "wk", "wv")
        }
        xt_sb = xpool.tile([P, NCC * T], bf16)

        def xdma(eng, n2, c):
            eng.dma_start(
                xt_sb[:, c * T + n2 * 1024 : c * T + (n2 + 1) * 1024],
                xT[c * P : (c + 1) * P, n2 * 1024 : (n2 + 1) * 1024],
            )

        nc.scalar.dma_start(w_sb["wq"], wq)
        xdma(nc.sync, 0, 0)
        nc.sync.dma_start(w_sb["wk"], wk)
        nc.gpsimd.dma_start(w_sb["wv"], wv)
        for n2 in range(2):
            for c in range(NCC):
                if n2 == 0 and c == 0:
                    continue
                xdma(nc.sync if (c + n2) % 2 == 0 else nc.scalar, n2, c)

        consts = ctx.enter_context(tc.tile_pool(name="consts", bufs=1))
        ones512 = consts.tile([P, QB], bf16)
        nc.vector.memset(ones512, 1.0)
        # additive causal mask for the last pair: slice [384:384+w] fills
        # -30000 where local col f < partition s_l
        MW = QB + 384
        mbig = consts.tile([P, MW], f32)
        nc.gpsimd.memset(mbig, 0.0)
        nc.gpsimd.affine_select(
            out=mbig,
            in_=mbig,
            compare_op=mybir.AluOpType.is_ge,
            fill=-30000.0,
            base=-384,
            pattern=[[1, MW]],
            channel_multiplier=-1,
        )
        prime = consts.tile([P, P], f32)
        nc.vector.tensor_copy(prime, mbig[:, :P])

        qkv = ctx.enter_context(tc.tile_pool(name="qkv", bufs=1))
        qkt_sb = qkv.tile([P, 2 * T], bf16)  # [:, :T] = QT, [:, T:] = KT
        qt_sb = qkt_sb[:, :T]
        kt_sb = qkt_sb[:, T:]
        vt_sb = qkv.tile([P, T], bf16)
        v_sb = qkv.tile([P, NT * H], bf16)
        v3 = v_sb.rearrange("p (t h) -> p t h", t=NT)

        # PSUM: att pairs 2x[128,1024] = 4 banks, pav 2, small 2
        ps_pair = ctx.enter_context(tc.tile_pool(name="ps_pair", bufs=2, space="PSUM"))
        ps_av = ctx.enter_context(tc.tile_pool(name="ps_av", bufs=2, space="PSUM"))
        ps_small = ctx.enter_context(
            tc.tile_pool(name="ps_small", bufs=2, space="PSUM")
        )

        # --- PE warmup: ramp p-state while DMA lands (results discarded) ---
        for i in range(9):
            pv = ps_small.tile([P, QB], f32, name=f"warm{i}", tag="small")
            nc.tensor.matmul(pv, ones512[:, :P], ones512, start=True, stop=True)

        def emit_proj_chunk(n, pairqk):
            xs = lambda c: xt_sb[:, c * T + n * QB : c * T + (n + 1) * QB]
            if pairqk:
                pt = ps_pair.tile([P, 1024], f32, name=f"pqk{n}", tag="pair")
                pv = ps_small.tile([P, QB], f32, name=f"pv{n}", tag="small")
                accs = {"wq": pt[:, :QB], "wk": pt[:, QB:], "wv": pv}
                loop = (
                    [(w, c) for c in range(NCC) for w in ("wq", "wk", "wv")]
                    if n == 0
                    else [(w, c) for w in ("wq", "wk", "wv") for c in range(NCC)]
                )
                for wname, c in loop:
                    nc.tensor.matmul(
                        accs[wname],
                        w_sb[wname][:, c * H : (c + 1) * H],
                        xs(c),
                        start=(c == 0),
                        stop=(c == NCC - 1),
                    )
                dst = qkt_sb.rearrange("p (two t) -> p two t", two=2)[
                    :, :, n * QB : (n + 1) * QB
                ]
                nc.scalar.activation(
                    dst,
                    pt.rearrange("p (two q) -> p two q", two=2),
                    mybir.ActivationFunctionType.Copy,
                )
                nc.vector.tensor_copy(vt_sb[:, n * QB : (n + 1) * QB], pv)
            nc.sync.dma_start(
                v3[:, n * 4 : (n + 1) * 4, :],
                vt_sb[:, n * QB : (n + 1) * QB],
                transpose=True,
            )

        def emit_proj_one(n, wname):
            """Single-projection unit for chunks interleaved with attention."""
            dst = {"wq": qt_sb, "wk": kt_sb, "wv": vt_sb}[wname]
            acc = ps_small.tile([P, QB], f32, name=f"p{wname}{n}", tag="small")
            for c in range(NCC):
                nc.tensor.matmul(
                    acc,
                    w_sb[wname][:, c * H : (c + 1) * H],
                    xt_sb[:, c * T + n * QB : c * T + (n + 1) * QB],
                    start=(c == 0),
                    stop=(c == NCC - 1),
                )
            if wname == "wv":
                nc.vector.tensor_copy(dst[:, n * QB : (n + 1) * QB], acc)
                nc.sync.dma_start(
                    v3[:, n * 4 : (n + 1) * 4, :],
                    vt_sb[:, n * QB : (n + 1) * QB],
                    transpose=True,
                )
            else:
                nc.scalar.activation(
                    dst[:, n * QB : (n + 1) * QB],
                    acc,
                    mybir.ActivationFunctionType.Copy,
                )

        # --- attention ---
        expst_pool = ctx.enter_context(tc.tile_pool(name="expst_pool", bufs=6))
        sacc_pool = ctx.enter_context(tc.tile_pool(name="sacc_pool", bufs=2))
        tp_pool = ctx.enter_context(tc.tile_pool(name="tp_pool", bufs=2))
        sbf_pool = ctx.enter_context(tc.tile_pool(name="sbf_pool", bufs=2))
        rec_pool = ctx.enter_context(tc.tile_pool(name="rec_pool", bufs=2))
        outp = ctx.enter_context(tc.tile_pool(name="outp", bufs=2))

        state = {}  # per-g: pav, sacc, then sbf
        GL = NQB - 1
        LAST = (GL, 2 * GL + 1)

        def tile_geom(g, p, k):
            """-> (d, col0, width, psum offset) for slice k of pair (g,p)."""
            j = 2 * p + k
            d = j - 4 * g
            if 0 <= d <= 3:
                w = QB - 128 * d
                # keep start=True slices in separate PSUM banks
                off = 0 if k == 0 else QB
                return d, 128 * d, w, off
            return None, 0, QB, k * QB

        def emit_scores(g, p):
            qs0 = g * QB
            ps = ps_pair.tile([P, 1024], f32, name=f"ps_{g}_{p}", tag="pair")
            geom = [tile_geom(g, p, k) for k in range(2)]
            for k in range(2):
                j = 2 * p + k
                d, col0, w, off = geom[k]
                nc.tensor.matmul(
                    ps[:, off : off + w],
                    kt_sb[:, j * P : (j + 1) * P],
                    qt_sb[:, qs0 + col0 : qs0 + QB],
                    start=True,
                    stop=True,
                )
            if (g, p) == LAST:
                for k in range(2):
                    d, col0, w, off = geom[k]
                    nc.vector.tensor_add(
                        ps[:, off : off + w],
                        ps[:, off : off + w],
                        mbig[:, 384 : 384 + w],
                    )
            w0 = geom[0][2]
            w1 = geom[1][2]
            expst = expst_pool.tile([P, 1024], bf16, name=f"e{g}_{p}", tag="expst")
            if geom[1][3] == w0:  # contiguous psum: single exp
                nc.scalar.activation(
                    expst[:, : w0 + w1],
                    ps[:, : w0 + w1],
                    mybir.ActivationFunctionType.Exp,
                )
            else:
                nc.scalar.activation(
                    expst[:, :w0], ps[:, :w0], mybir.ActivationFunctionType.Exp
                )
                nc.scalar.activation(
                    expst[:, w0 : w0 + w1],
                    ps[:, QB : QB + w1],
                    mybir.ActivationFunctionType.Exp,
                )
            if (g, p) != LAST:
                for k in range(2):
                    d, col0, w, off = geom[k]
                    if d is not None:
                        e0 = 0 if k == 0 else w0
                        nc.gpsimd.affine_select(
                            out=expst[:, e0 : e0 + w],
                            in_=expst[:, e0 : e0 + w],
                            compare_op=mybir.AluOpType.is_ge,
                            fill=0.0,
                            base=0,
                            pattern=[[1, w]],
                            channel_multiplier=-1,
                        )
            return expst

        def emit_av(g, p, expst):
            npair = 2 * g + 2
            njt = 4 * g + 4
            if p == 0:
                state[g] = [
                    ps_av.tile([P, QB], f32, name=f"pav{g}", tag="pav"),
                    sacc_pool.tile([P, QB], f32, name=f"sacc{g}", tag="sacc"),
                    None,
                ]
            pav, sacc = state[g][0], state[g][1]
            geom = [tile_geom(g, p, k) for k in range(2)]
            w0 = geom[0][2]
            eslice = [expst[:, :w0], expst[:, w0 : w0 + geom[1][2]]]
            for k in range(2):
                j = 2 * p + k
                d, col0, w, off = geom[k]
                nc.tensor.matmul(
                    pav[:, col0:QB],
                    v_sb[:, j * H : (j + 1) * H],
                    eslice[k],
                    start=(j == 0),
                    stop=(j == njt - 1),
                )
            if g == GL and p == npair - 2:
                # row-sums for the last two pairs finish on PE: partial
                # ones-matmul over the DVE-accumulated pairs 0..npair-3, then
                # this pair's two slices
                psum = ps_small.tile([P, QB], f32, name=f"psum{g}", tag="small")
                state[g].append(psum)
                nc.tensor.matmul(
                    psum, ones512[:, :P], state[g][2], start=True, stop=False
                )
                for k in range(2):
                    d, col0, w, off = geom[k]
                    nc.tensor.matmul(
                        psum[:, col0:QB],
                        ones512[:, :P],
                        eslice[k],
                        start=False,
                        stop=False,
                    )
                return
            if (g, p) == LAST:
                psum = state[g][3]
                for k in range(2):
                    d, col0, w, off = geom[k]
                    nc.tensor.matmul(
                        psum[:, col0:QB],
                        ones512[:, :P],
                        eslice[k],
                        start=False,
                        stop=(k == 1),
                    )
                qs0 = g * QB
                for h in range(2):
                    hs = slice(h * 256, (h + 1) * 256)
                    rec = rec_pool.tile([P, QB], f32, name=f"rec{g}{h}", tag="rec")
                    nc.vector.reciprocal_approx_fast(rec[:, :256], psum[:, hs])
                    o = outp.tile([P, QB], f32, name=f"o{g}{h}", tag="o")
                    nc.vector.tensor_mul(o[:, :256], pav[:, hs], rec[:, :256])
                    eng = nc.sync if h == 0 else nc.scalar
                    eng.dma_start(
                        outT[:, qs0 + h * 256 : qs0 + (h + 1) * 256], o[:, :256]
                    )
                return
            # row-sum accumulation: GpSimd folds, DVE accumulates
            if geom[0][0] is None and geom[1][0] is None:
                if p == 0:
                    nc.vector.tensor_add(sacc, eslice[0], eslice[1])
                else:
                    tp = tp_pool.tile([P, QB], bf16, name=f"tp{g}_{p}", tag="tp")
                    nc.vector.tensor_add(tp, eslice[0], eslice[1])
                    nc.vector.tensor_add(sacc, sacc, tp)
            else:
                for k in range(2):
                    d, col0, w, off = geom[k]
                    if p == 0 and k == 0:
                        nc.vector.tensor_copy(sacc[:, col0:QB], eslice[k])
                    else:
                        nc.vector.tensor_add(
                            sacc[:, col0:QB], sacc[:, col0:QB], eslice[k]
                        )
            if (g != GL and p == npair - 1) or (g == GL and p == npair - 3):
                sbf = sbf_pool.tile([P, QB], bf16, name=f"sbf{g}", tag="sbf")
                nc.vector.tensor_copy(sbf, sacc)
                state[g][2] = sbf

        def emit_ep(g):
            qs0 = g * QB
            psum = ps_small.tile([P, QB], f32, name=f"psum{g}", tag="small")
            nc.tensor.matmul(psum, ones512[:, :P], state[g][2], start=True, stop=True)
            rec = rec_pool.tile([P, QB], f32, name=f"rec{g}", tag="rec")
            nc.vector.reciprocal_approx_fast(rec, psum)
            o = outp.tile([P, QB], f32, name=f"o{g}", tag="o")
            nc.vector.tensor_mul(o, state[g][0], rec)
            nc.sync.dma_start(outT[:, qs0 : qs0 + QB], o)

        # --- emission order: proj chunks 2,3 split per-projection and
        # slotted between attention pairs so the PE hides exp latency ---
        S, AV, PC, P1, EP = "S", "AV", "P", "p", "E"
        order = [
            (PC, 0), (PC, 1), (S, 0, 0), (S, 0, 1), (P1, 2, "wq"), (AV, 0, 0),
            (P1, 2, "wk"), (AV, 0, 1), (P1, 2, "wv"), (S, 1, 0), (S, 1, 1),
            (P1, 3, "wq"), (AV, 1, 0), (P1, 3, "wk"), (EP, 0), (AV, 1, 1),
            (S, 1, 2), (P1, 3, "wv"), (S, 1, 3), (AV, 1, 2), (S, 2, 0),
            (AV, 1, 3), (S, 2, 1), (EP, 1), (AV, 2, 0), (S, 2, 2), (AV, 2, 1),
            (S, 2, 3), (AV, 2, 2), (S, 2, 4), (AV, 2, 3), (S, 2, 5),
            (AV, 2, 4), (S, 3, 0), (AV, 2, 5), (S, 3, 1), (EP, 2), (AV, 3, 0),
            (S, 3, 2), (AV, 3, 1), (S, 3, 3), (AV, 3, 2), (S, 3, 4),
            (AV, 3, 3), (S, 3, 5), (AV, 3, 4), (S, 3, 6), (AV, 3, 5),
            (S, 3, 7), (AV, 3, 6), (AV, 3, 7),
        ]

        exps = {}
        for u in order:
            if u[0] == PC:
                emit_proj_chunk(u[1], pairqk=True)
            elif u[0] == P1:
                emit_proj_one(u[1], u[2])
            elif u[0] == S:
                exps[(u[1], u[2])] = emit_scores(u[1], u[2])
            elif u[0] == EP:
                emit_ep(u[1])
            else:
                emit_av(u[1], u[2], exps.pop((u[1], u[2])))

    nc.compile()
    return nc


def _get_bass():
    if "k" not in _CACHE:
        _CACHE["k"] = _build()
    return _CACHE["k"]


LAST_RESULT = None  # BassKernelResults of the most recent kernel() call


def kernel(x, Wq, Wk, Wv):
    global LAST_RESULT
    from concourse.bass_utils import run_bass_kernel_spmd

    np_dt = ml_dtypes.bfloat16

    def _wlayout(w):  # [C, H] -> [P, NCC*H]: sbuf layout, contiguous DMA
        w = np.asarray(w, np.float32).reshape(NCC, P, H).transpose(1, 0, 2)
        return np.ascontiguousarray(w.reshape(P, NCC * H)).astype(np_dt)

    wq_s = _wlayout(np.asarray(Wq, np.float32) * SCALE)
    wk_s = _wlayout(Wk)
    wv_s = _wlayout(Wv)
    x = np.asarray(x, np.float32)

    in_maps = []
    for b in range(N_CORES):
        in_maps.append(
            {
                "xT": np.ascontiguousarray(x[b].T).astype(np_dt),
                "wq": wq_s,
                "wk": wk_s,
                "wv": wv_s,
            }
        )

    nc = _get_bass()
    res = run_bass_kernel_spmd(nc, in_maps, core_ids=list(range(N_CORES)))
    LAST_RESULT = res
    return np.stack(
        [np.ascontiguousarray(r["outT"].T) for r in res.results], axis=0
    )


# revision 31
# speedup vs baseline: 1.1655x; 1.0481x over previous
"""Single-head causal attention (B=8, T=2048, C=1024, H=128) on 8 TRN2 NeuronCores.

Sharding: data-parallel over batch B — core b computes attention for x[b].
Host-side prep per core: x[b] is transposed to xT [C, T] (contraction dim C on
SBUF partitions) and the softmax scale C**-0.5 is folded into Wq. The kernel
computes in the transposed orientation; the host untransposes the [H, T] output.

Device kernel per core (ST-direct, all bf16 matmuls):
  warmup: junk matmuls ramp the PE p-state while the x DMA lands.
  projections per 512-col n-chunk; chunk 0 interleaves Q/K/V per c-chunk so
  the PE chases the x DMA arrival order. Q,K accumulate in one [128,1024]
  PSUM pair tile (chunks 0,1) or two single banks (chunks 2,3 — those
  interleave with attention, which owns the pair pool); one ACT copy per
  chunk moves Q,K to SBUF, DVE moves V, and a DMA-xbar transpose yields
  V [T,H] tiles per chunk.
  attention, software pipeline over (g, pair), scores one pair ahead of AV,
  with projection chunks 2,3 slotted between the first pairs:
    ST_jg = KT_j.T @ QT_g   [128 s, <=512 q] x2 per pair  (PE)
    diagonal tiles are column-trimmed to 512/384/256/128 (causally dead
    columns never computed), masked post-exp via affine_select (GpSimd)
    expST = exp(ST)         -> SBUF bf16                  (ACT)
    row-sums: GpSimd folds e0+e1, DVE accumulates into f32 (not PE)
    outT_g += V_j.T @ expST [H, 512] PSUM                 (PE)
  per g epilogue (own unit, placed 2+ units later so the PE never waits on
  DVE): ones-matmul broadcasts the partition row-sum, DVE reciprocal +
  multiply, DMA out on the sync queue. The last pair of the last q-block
  takes a short-latency path: additive -30000 mask pre-exp (DVE), row-sums
  finished by PE ones-matmuls on the exp tile, halved rec/mul/DMA epilogue
  on two queues.
"""

import os
from contextlib import ExitStack

import numpy as np
import ml_dtypes

B, T, C, H = 8, 2048, 1024, 128
P = 128
NT = T // P  # 16 s-tiles per core
NCC = C // P  # 8 contraction chunks
QB = 512  # q-block width
NQB = T // QB  # 4 q-blocks
N_CORES = 8
SCALE = float(C) ** -0.5

_CACHE = {}


def _build():
    import concourse.bass as bass  # noqa: F401
    import concourse.mybir as mybir
    import concourse.tile as tile
    from concourse import bacc

    dt = mybir.dt
    bf16 = dt.bfloat16
    f32 = dt.float32

    nc = bacc.Bacc("TRN2", target_bir_lowering=False, debug=False)
    xT = nc.dram_tensor("xT", [C, T], bf16, kind="ExternalInput").ap()
    wq = nc.dram_tensor("wq", [P, NCC * H], bf16, kind="ExternalInput").ap()
    wk = nc.dram_tensor("wk", [P, NCC * H], bf16, kind="ExternalInput").ap()
    wv = nc.dram_tensor("wv", [P, NCC * H], bf16, kind="ExternalInput").ap()
    outT = nc.dram_tensor("outT", [H, T], f32, kind="ExternalOutput").ap()

    with tile.TileContext(nc) as tc, ExitStack() as ctx:
        # --- input DMA: wq leads scalar while x c0 leads sync so the first
        # matmul's two dependencies arrive in parallel; wv on gpsimd ---
        wpool = ctx.enter_context(tc.tile_pool(name="wpool", bufs=1))
        xpool = ctx.enter_context(tc.tile_pool(name="xpool", bufs=1))
        w_sb = {
            name: wpool.tile([P, NCC * H], bf16, name=f"{name}_sb")
            for name in ("wq", "wk", "wv")
        }
        xt_sb = xpool.tile([P, NCC * T], bf16)

        def xdma(eng, n2, c):
            eng.dma_start(
                xt_sb[:, c * T + n2 * 1024 : c * T + (n2 + 1) * 1024],
                xT[c * P : (c + 1) * P, n2 * 1024 : (n2 + 1) * 1024],
            )

        nc.scalar.dma_start(w_sb["wq"], wq)
        xdma(nc.sync, 0, 0)
        nc.sync.dma_start(w_sb["wk"], wk)
        nc.gpsimd.dma_start(w_sb["wv"], wv)
        for n2 in range(2):
            for c in range(NCC):
                if n2 == 0 and c == 0:
                    continue
                xdma(nc.sync if (c + n2) % 2 == 0 else nc.scalar, n2, c)

        consts = ctx.enter_context(tc.tile_pool(name="consts", bufs=1))
        ones512 = consts.tile([P, QB], bf16)
        nc.vector.memset(ones512, 1.0)
        # additive causal mask for the last pair: slice [384:384+w] fills
        # -30000 where local col f < partition s_l
        MW = QB + 384
        mbig = consts.tile([P, MW], f32)
        nc.gpsimd.memset(mbig, 0.0)
        nc.gpsimd.affine_select(
            out=mbig,
            in_=mbig,
            compare_op=mybir.AluOpType.is_ge,
            fill=-30000.0,
            base=-384,
            pattern=[[1, MW]],
            channel_multiplier=-1,
        )
        prime = consts.tile([P, P], f32)
        nc.vector.tensor_copy(prime, mbig[:, :P])

        qkv = ctx.enter_context(tc.tile_pool(name="qkv", bufs=1))
        qkt_sb = qkv.tile([P, 2 * T], bf16)  # [:, :T] = QT, [:, T:] = KT
        qt_sb = qkt_sb[:, :T]
        kt_sb = qkt_sb[:, T:]
        vt_sb = qkv.tile([P, T], bf16)
        v_sb = qkv.tile([P, NT * H], bf16)
        v3 = v_sb.rearrange("p (t h) -> p t h", t=NT)

        # PSUM: att pairs 2x[128,1024] = 4 banks, pav 2, small 2
        ps_pair = ctx.enter_context(tc.tile_pool(name="ps_pair", bufs=2, space="PSUM"))
        ps_av = ctx.enter_context(tc.tile_pool(name="ps_av", bufs=2, space="PSUM"))
        ps_small = ctx.enter_context(
            tc.tile_pool(name="ps_small", bufs=2, space="PSUM")
        )

        # --- PE warmup: ramp p-state while DMA lands (results discarded) ---
        for i in range(9):
            pv = ps_small.tile([P, QB], f32, name=f"warm{i}", tag="small")
            nc.tensor.matmul(pv, ones512[:, :P], ones512, start=True, stop=True)

        def emit_proj_chunk(n, pairqk):
            xs = lambda c: xt_sb[:, c * T + n * QB : c * T + (n + 1) * QB]
            if pairqk:
                pt = ps_pair.tile([P, 1024], f32, name=f"pqk{n}", tag="pair")
                pv = ps_small.tile([P, QB], f32, name=f"pv{n}", tag="small")
                accs = {"wq": pt[:, :QB], "wk": pt[:, QB:], "wv": pv}
                loop = (
                    [(w, c) for c in range(NCC) for w in ("wq", "wk", "wv")]
                    if n == 0
                    else [(w, c) for w in ("wq", "wk", "wv") for c in range(NCC)]
                )
                for wname, c in loop:
                    nc.tensor.matmul(
                        accs[wname],
                        w_sb[wname][:, c * H : (c + 1) * H],
                        xs(c),
                        start=(c == 0),
                        stop=(c == NCC - 1),
                    )
                dst = qkt_sb.rearrange("p (two t) -> p two t", two=2)[
                    :, :, n * QB : (n + 1) * QB
                ]
                nc.scalar.activation(
                    dst,
                    pt.rearrange("p (two q) -> p two q", two=2),
                    mybir.ActivationFunctionType.Copy,
                )
                nc.vector.tensor_copy(vt_sb[:, n * QB : (n + 1) * QB], pv)
            nc.sync.dma_start(
                v3[:, n * 4 : (n + 1) * 4, :],
                vt_sb[:, n * QB : (n + 1) * QB],
                transpose=True,
            )

        def emit_proj_one(n, wname):
            """Single-projection unit for chunks interleaved with attention."""
            dst = {"wq": qt_sb, "wk": kt_sb, "wv": vt_sb}[wname]
            acc = ps_small.tile([P, QB], f32, name=f"p{wname}{n}", tag="small")
            for c in range(NCC):
                nc.tensor.matmul(
                    acc,
                    w_sb[wname][:, c * H : (c + 1) * H],
                    xt_sb[:, c * T + n * QB : c * T + (n + 1) * QB],
                    start=(c == 0),
                    stop=(c == NCC - 1),
                )
            if wname == "wv":
                nc.vector.tensor_copy(dst[:, n * QB : (n + 1) * QB], acc)
                nc.sync.dma_start(
                    v3[:, n * 4 : (n + 1) * 4, :],
                    vt_sb[:, n * QB : (n + 1) * QB],
                    transpose=True,
                )
            else:
                # DVE, not ACT: a copy on ACT would sit between exps in
                # program order and delay the attention exp stream
                nc.vector.tensor_copy(dst[:, n * QB : (n + 1) * QB], acc)

        # --- attention ---
        expst_pool = ctx.enter_context(tc.tile_pool(name="expst_pool", bufs=6))
        sacc_pool = ctx.enter_context(tc.tile_pool(name="sacc_pool", bufs=2))
        tp_pool = ctx.enter_context(tc.tile_pool(name="tp_pool", bufs=2))
        sbf_pool = ctx.enter_context(tc.tile_pool(name="sbf_pool", bufs=2))
        rec_pool = ctx.enter_context(tc.tile_pool(name="rec_pool", bufs=2))
        outp = ctx.enter_context(tc.tile_pool(name="outp", bufs=2))

        state = {}  # per-g: pav, sacc, then sbf
        GL = NQB - 1
        LAST = (GL, 2 * GL + 1)

        def tile_geom(g, p, k):
            """-> (d, col0, width, psum offset) for slice k of pair (g,p)."""
            j = 2 * p + k
            d = j - 4 * g
            if 0 <= d <= 3:
                w = QB - 128 * d
                # keep start=True slices in separate PSUM banks
                off = 0 if k == 0 else QB
                return d, 128 * d, w, off
            return None, 0, QB, k * QB

        def emit_scores(g, p):
            qs0 = g * QB
            ps = ps_pair.tile([P, 1024], f32, name=f"ps_{g}_{p}", tag="pair")
            geom = [tile_geom(g, p, k) for k in range(2)]
            for k in range(2):
                j = 2 * p + k
                d, col0, w, off = geom[k]
                nc.tensor.matmul(
                    ps[:, off : off + w],
                    kt_sb[:, j * P : (j + 1) * P],
                    qt_sb[:, qs0 + col0 : qs0 + QB],
                    start=True,
                    stop=True,
                )
            if (g, p) == LAST:
                for k in range(2):
                    d, col0, w, off = geom[k]
                    nc.vector.tensor_add(
                        ps[:, off : off + w],
                        ps[:, off : off + w],
                        mbig[:, 384 : 384 + w],
                    )
            w0 = geom[0][2]
            w1 = geom[1][2]
            expst = expst_pool.tile([P, 1024], bf16, name=f"e{g}_{p}", tag="expst")
            if geom[1][3] == w0:  # contiguous psum: single exp
                nc.scalar.activation(
                    expst[:, : w0 + w1],
                    ps[:, : w0 + w1],
                    mybir.ActivationFunctionType.Exp,
                )
            else:
                nc.scalar.activation(
                    expst[:, :w0], ps[:, :w0], mybir.ActivationFunctionType.Exp
                )
                nc.scalar.activation(
                    expst[:, w0 : w0 + w1],
                    ps[:, QB : QB + w1],
                    mybir.ActivationFunctionType.Exp,
                )
            if (g, p) != LAST:
                for k in range(2):
                    d, col0, w, off = geom[k]
                    if d is not None:
                        e0 = 0 if k == 0 else w0
                        nc.gpsimd.affine_select(
                            out=expst[:, e0 : e0 + w],
                            in_=expst[:, e0 : e0 + w],
                            compare_op=mybir.AluOpType.is_ge,
                            fill=0.0,
                            base=0,
                            pattern=[[1, w]],
                            channel_multiplier=-1,
                        )
            return expst

        def emit_av(g, p, expst):
            npair = 2 * g + 2
            njt = 4 * g + 4
            if p == 0:
                state[g] = [
                    ps_av.tile([P, QB], f32, name=f"pav{g}", tag="pav"),
                    sacc_pool.tile([P, QB], f32, name=f"sacc{g}", tag="sacc"),
                    None,
                ]
            pav, sacc = state[g][0], state[g][1]
            geom = [tile_geom(g, p, k) for k in range(2)]
            w0 = geom[0][2]
            eslice = [expst[:, :w0], expst[:, w0 : w0 + geom[1][2]]]
            for k in range(2):
                j = 2 * p + k
                d, col0, w, off = geom[k]
                nc.tensor.matmul(
                    pav[:, col0:QB],
                    v_sb[:, j * H : (j + 1) * H],
                    eslice[k],
                    start=(j == 0),
                    stop=(j == njt - 1),
                )
            if g == GL and p == npair - 2:
                # row-sums for the last two pairs finish on PE: partial
                # ones-matmul over the DVE-accumulated pairs 0..npair-3, then
                # this pair's two slices
                psum = ps_small.tile([P, QB], f32, name=f"psum{g}", tag="small")
                state[g].append(psum)
                nc.tensor.matmul(
                    psum, ones512[:, :P], state[g][2], start=True, stop=False
                )
                for k in range(2):
                    d, col0, w, off = geom[k]
                    nc.tensor.matmul(
                        psum[:, col0:QB],
                        ones512[:, :P],
                        eslice[k],
                        start=False,
                        stop=False,
                    )
                return
            if (g, p) == LAST:
                psum = state[g][3]
                for k in range(2):
                    d, col0, w, off = geom[k]
                    nc.tensor.matmul(
                        psum[:, col0:QB],
                        ones512[:, :P],
                        eslice[k],
                        start=False,
                        stop=(k == 1),
                    )
                qs0 = g * QB
                for h in range(2):
                    hs = slice(h * 256, (h + 1) * 256)
                    rec = rec_pool.tile([P, QB], f32, name=f"rec{g}{h}", tag="rec")
                    nc.vector.reciprocal_approx_fast(rec[:, :256], psum[:, hs])
                    o = outp.tile([P, QB], f32, name=f"o{g}{h}", tag="o")
                    nc.vector.tensor_mul(o[:, :256], pav[:, hs], rec[:, :256])
                    eng = nc.sync if h == 0 else nc.scalar
                    eng.dma_start(
                        outT[:, qs0 + h * 256 : qs0 + (h + 1) * 256], o[:, :256]
                    )
                return
            # row-sum accumulation: GpSimd folds, DVE accumulates
            if geom[0][0] is None and geom[1][0] is None:
                if p == 0:
                    nc.vector.tensor_add(sacc, eslice[0], eslice[1])
                else:
                    tp = tp_pool.tile([P, QB], bf16, name=f"tp{g}_{p}", tag="tp")
                    nc.vector.tensor_add(tp, eslice[0], eslice[1])
                    nc.vector.tensor_add(sacc, sacc, tp)
            else:
                for k in range(2):
                    d, col0, w, off = geom[k]
                    if p == 0 and k == 0:
                        nc.vector.tensor_copy(sacc[:, col0:QB], eslice[k])
                    else:
                        nc.vector.tensor_add(
                            sacc[:, col0:QB], sacc[:, col0:QB], eslice[k]
                        )
            if (g != GL and p == npair - 1) or (g == GL and p == npair - 3):
                sbf = sbf_pool.tile([P, QB], bf16, name=f"sbf{g}", tag="sbf")
                nc.vector.tensor_copy(sbf, sacc)
                state[g][2] = sbf

        def emit_ep(g):
            qs0 = g * QB
            psum = ps_small.tile([P, QB], f32, name=f"psum{g}", tag="small")
            nc.tensor.matmul(psum, ones512[:, :P], state[g][2], start=True, stop=True)
            rec = rec_pool.tile([P, QB], f32, name=f"rec{g}", tag="rec")
            nc.vector.reciprocal_approx_fast(rec, psum)
            o = outp.tile([P, QB], f32, name=f"o{g}", tag="o")
            nc.vector.tensor_mul(o, state[g][0], rec)
            nc.sync.dma_start(outT[:, qs0 : qs0 + QB], o)

        # --- emission order: proj chunks 2,3 split per-projection and
        # slotted between attention pairs so the PE hides exp latency ---
        S, AV, PC, P1, EP = "S", "AV", "P", "p", "E"
        order = [
            (PC, 0), (PC, 1), (S, 0, 0), (S, 0, 1), (P1, 2, "wq"), (AV, 0, 0),
            (P1, 2, "wk"), (AV, 0, 1), (P1, 2, "wv"), (S, 1, 0), (S, 1, 1),
            (P1, 3, "wq"), (AV, 1, 0), (P1, 3, "wk"), (EP, 0), (AV, 1, 1),
            (S, 1, 2), (P1, 3, "wv"), (S, 1, 3), (AV, 1, 2), (S, 2, 0),
            (AV, 1, 3), (S, 2, 1), (EP, 1), (AV, 2, 0), (S, 2, 2), (AV, 2, 1),
            (S, 2, 3), (AV, 2, 2), (S, 2, 4), (AV, 2, 3), (S, 2, 5),
            (AV, 2, 4), (S, 3, 0), (AV, 2, 5), (S, 3, 1), (EP, 2), (AV, 3, 0),
            (S, 3, 2), (AV, 3, 1), (S, 3, 3), (AV, 3, 2), (S, 3, 4),
            (AV, 3, 3), (S, 3, 5), (AV, 3, 4), (S, 3, 6), (AV, 3, 5),
            (S, 3, 7), (AV, 3, 6), (AV, 3, 7),
        ]

        exps = {}
        for u in order:
            if u[0] == PC:
                emit_proj_chunk(u[1], pairqk=True)
            elif u[0] == P1:
                emit_proj_one(u[1], u[2])
            elif u[0] == S:
                exps[(u[1], u[2])] = emit_scores(u[1], u[2])
            elif u[0] == EP:
                emit_ep(u[1])
            else:
                emit_av(u[1], u[2], exps.pop((u[1], u[2])))

    nc.compile()
    return nc


def _get_bass():
    if "k" not in _CACHE:
        _CACHE["k"] = _build()
    return _CACHE["k"]


LAST_RESULT = None  # BassKernelResults of the most recent kernel() call


def kernel(x, Wq, Wk, Wv):
    global LAST_RESULT
    from concourse.bass_utils import run_bass_kernel_spmd

    np_dt = ml_dtypes.bfloat16

    def _wlayout(w):  # [C, H] -> [P, NCC*H]: sbuf layout, contiguous DMA
        w = np.asarray(w, np.float32).reshape(NCC, P, H).transpose(1, 0, 2)
        return np.ascontiguousarray(w.reshape(P, NCC * H)).astype(np_dt)

    wq_s = _wlayout(np.asarray(Wq, np.float32) * SCALE)
    wk_s = _wlayout(Wk)
    wv_s = _wlayout(Wv)
    x = np.asarray(x, np.float32)

    in_maps = []
    for b in range(N_CORES):
        in_maps.append(
            {
                "xT": np.ascontiguousarray(x[b].T).astype(np_dt),
                "wq": wq_s,
                "wk": wk_s,
                "wv": wv_s,
            }
        )

    nc = _get_bass()
    res = run_bass_kernel_spmd(nc, in_maps, core_ids=list(range(N_CORES)))
    LAST_RESULT = res
    return np.stack(
        [np.ascontiguousarray(r["outT"].T) for r in res.results], axis=0
    )


# revision 32
# speedup vs baseline: 1.1777x; 1.0104x over previous
"""Single-head causal attention (B=8, T=2048, C=1024, H=128) on 8 TRN2 NeuronCores.

Sharding: data-parallel over batch B — core b computes attention for x[b].
Host-side prep per core: x[b] is transposed to xT [C, T] (contraction dim C on
SBUF partitions) and the softmax scale C**-0.5 is folded into Wq. The kernel
computes in the transposed orientation; the host untransposes the [H, T] output.

Device kernel per core (ST-direct, all bf16 matmuls):
  warmup: junk matmuls ramp the PE p-state while the x DMA lands.
  projections per 512-col n-chunk; chunk 0 interleaves Q/K/V per c-chunk so
  the PE chases the x DMA arrival order. Q,K accumulate in one [128,1024]
  PSUM pair tile (chunks 0,1) or two single banks (chunks 2,3 — those
  interleave with attention, which owns the pair pool); one ACT copy per
  chunk moves Q,K to SBUF, DVE moves V, and a DMA-xbar transpose yields
  V [T,H] tiles per chunk.
  attention, software pipeline over (g, pair), scores one pair ahead of AV,
  with projection chunks 2,3 slotted between the first pairs:
    ST_jg = KT_j.T @ QT_g   [128 s, <=512 q] x2 per pair  (PE)
    diagonal tiles are column-trimmed to 512/384/256/128 (causally dead
    columns never computed), masked post-exp via affine_select (GpSimd)
    expST = exp(ST)         -> SBUF bf16                  (ACT)
    row-sums: GpSimd folds e0+e1, DVE accumulates into f32 (not PE)
    outT_g += V_j.T @ expST [H, 512] PSUM                 (PE)
  per g epilogue (own unit, placed 2+ units later so the PE never waits on
  DVE): ones-matmul broadcasts the partition row-sum, DVE reciprocal +
  multiply, DMA out on the sync queue. The last pair of the last q-block
  takes a short-latency path: additive -30000 mask pre-exp (DVE), row-sums
  finished by PE ones-matmuls on the exp tile, halved rec/mul/DMA epilogue
  on two queues.
"""

import os
from contextlib import ExitStack

import numpy as np
import ml_dtypes

B, T, C, H = 8, 2048, 1024, 128
P = 128
NT = T // P  # 16 s-tiles per core
NCC = C // P  # 8 contraction chunks
QB = 512  # q-block width
NQB = T // QB  # 4 q-blocks
N_CORES = 8
SCALE = float(C) ** -0.5

_CACHE = {}


def _build():
    import concourse.bass as bass  # noqa: F401
    import concourse.mybir as mybir
    import concourse.tile as tile
    from concourse import bacc

    dt = mybir.dt
    bf16 = dt.bfloat16
    f32 = dt.float32

    nc = bacc.Bacc("TRN2", target_bir_lowering=False, debug=False)
    xT = nc.dram_tensor("xT", [C, T], bf16, kind="ExternalInput").ap()
    wq = nc.dram_tensor("wq", [P, NCC * H], bf16, kind="ExternalInput").ap()
    wk = nc.dram_tensor("wk", [P, NCC * H], bf16, kind="ExternalInput").ap()
    wv = nc.dram_tensor("wv", [P, NCC * H], bf16, kind="ExternalInput").ap()
    outT = nc.dram_tensor("outT", [H, T], f32, kind="ExternalOutput").ap()

    with tile.TileContext(nc) as tc, ExitStack() as ctx:
        # --- input DMA: wq leads scalar while x c0 leads sync so the first
        # matmul's two dependencies arrive in parallel; wv on gpsimd ---
        wpool = ctx.enter_context(tc.tile_pool(name="wpool", bufs=1))
        xpool = ctx.enter_context(tc.tile_pool(name="xpool", bufs=1))
        w_sb = {
            name: wpool.tile([P, NCC * H], bf16, name=f"{name}_sb")
            for name in ("wq", "wk", "wv")
        }
        xt_sb = xpool.tile([P, NCC * T], bf16)

        def xdma(eng, n2, c):
            eng.dma_start(
                xt_sb[:, c * T + n2 * 1024 : c * T + (n2 + 1) * 1024],
                xT[c * P : (c + 1) * P, n2 * 1024 : (n2 + 1) * 1024],
            )

        nc.scalar.dma_start(w_sb["wq"], wq)
        xdma(nc.sync, 0, 0)
        nc.sync.dma_start(w_sb["wk"], wk)
        nc.gpsimd.dma_start(w_sb["wv"], wv)
        for n2 in range(2):
            for c in range(NCC):
                if n2 == 0 and c == 0:
                    continue
                xdma(nc.sync if (c + n2) % 2 == 0 else nc.scalar, n2, c)

        consts = ctx.enter_context(tc.tile_pool(name="consts", bufs=1))
        ones512 = consts.tile([P, QB], bf16)
        nc.vector.memset(ones512, 1.0)
        # additive causal mask for the last pair: slice [384:384+w] fills
        # -30000 where local col f < partition s_l
        MW = QB + 384
        mbig = consts.tile([P, MW], f32)
        nc.gpsimd.memset(mbig, 0.0)
        nc.gpsimd.affine_select(
            out=mbig,
            in_=mbig,
            compare_op=mybir.AluOpType.is_ge,
            fill=-30000.0,
            base=-384,
            pattern=[[1, MW]],
            channel_multiplier=-1,
        )
        prime = consts.tile([P, P], f32)
        nc.vector.tensor_copy(prime, mbig[:, :P])

        qkv = ctx.enter_context(tc.tile_pool(name="qkv", bufs=1))
        qkt_sb = qkv.tile([P, 2 * T], bf16)  # [:, :T] = QT, [:, T:] = KT
        qt_sb = qkt_sb[:, :T]
        kt_sb = qkt_sb[:, T:]
        vt_sb = qkv.tile([P, T], bf16)
        v_sb = qkv.tile([P, NT * H], bf16)
        v3 = v_sb.rearrange("p (t h) -> p t h", t=NT)

        # PSUM: att pairs 2x[128,1024] = 4 banks, pav 2, small 2
        ps_pair = ctx.enter_context(tc.tile_pool(name="ps_pair", bufs=2, space="PSUM"))
        ps_av = ctx.enter_context(tc.tile_pool(name="ps_av", bufs=2, space="PSUM"))
        ps_small = ctx.enter_context(
            tc.tile_pool(name="ps_small", bufs=2, space="PSUM")
        )

        # --- PE warmup: ramp p-state while DMA lands (results discarded) ---
        for i in range(9):
            pv = ps_small.tile([P, QB], f32, name=f"warm{i}", tag="small")
            nc.tensor.matmul(pv, ones512[:, :P], ones512, start=True, stop=True)

        def emit_proj_chunk(n, pairqk):
            xs = lambda c: xt_sb[:, c * T + n * QB : c * T + (n + 1) * QB]
            if pairqk:
                pt = ps_pair.tile([P, 1024], f32, name=f"pqk{n}", tag="pair")
                pv = ps_small.tile([P, QB], f32, name=f"pv{n}", tag="small")
                accs = {"wq": pt[:, :QB], "wk": pt[:, QB:], "wv": pv}
                loop = (
                    [(w, c) for c in range(NCC) for w in ("wq", "wk", "wv")]
                    if n == 0
                    else [(w, c) for w in ("wq", "wk", "wv") for c in range(NCC)]
                )
                for wname, c in loop:
                    nc.tensor.matmul(
                        accs[wname],
                        w_sb[wname][:, c * H : (c + 1) * H],
                        xs(c),
                        start=(c == 0),
                        stop=(c == NCC - 1),
                    )
                dst = qkt_sb.rearrange("p (two t) -> p two t", two=2)[
                    :, :, n * QB : (n + 1) * QB
                ]
                nc.scalar.activation(
                    dst,
                    pt.rearrange("p (two q) -> p two q", two=2),
                    mybir.ActivationFunctionType.Copy,
                )
                nc.vector.tensor_copy(vt_sb[:, n * QB : (n + 1) * QB], pv)
            nc.sync.dma_start(
                v3[:, n * 4 : (n + 1) * 4, :],
                vt_sb[:, n * QB : (n + 1) * QB],
                transpose=True,
            )

        def emit_proj_one(n, wname):
            """Single-projection unit for chunks interleaved with attention."""
            dst = {"wq": qt_sb, "wk": kt_sb, "wv": vt_sb}[wname]
            acc = ps_small.tile([P, QB], f32, name=f"p{wname}{n}", tag="small")
            for c in range(NCC):
                nc.tensor.matmul(
                    acc,
                    w_sb[wname][:, c * H : (c + 1) * H],
                    xt_sb[:, c * T + n * QB : c * T + (n + 1) * QB],
                    start=(c == 0),
                    stop=(c == NCC - 1),
                )
            if wname == "wv":
                nc.vector.tensor_copy(dst[:, n * QB : (n + 1) * QB], acc)
                nc.sync.dma_start(
                    v3[:, n * 4 : (n + 1) * 4, :],
                    vt_sb[:, n * QB : (n + 1) * QB],
                    transpose=True,
                )
            else:
                # DVE, not ACT: a copy on ACT would sit between exps in
                # program order and delay the attention exp stream
                nc.vector.tensor_copy(dst[:, n * QB : (n + 1) * QB], acc)

        # --- attention ---
        expst_pool = ctx.enter_context(tc.tile_pool(name="expst_pool", bufs=8))
        sacc_pool = ctx.enter_context(tc.tile_pool(name="sacc_pool", bufs=2))
        tp_pool = ctx.enter_context(tc.tile_pool(name="tp_pool", bufs=4))
        sbf_pool = ctx.enter_context(tc.tile_pool(name="sbf_pool", bufs=2))
        rec_pool = ctx.enter_context(tc.tile_pool(name="rec_pool", bufs=2))
        outp = ctx.enter_context(tc.tile_pool(name="outp", bufs=2))

        state = {}  # per-g: pav, sacc, then sbf
        GL = NQB - 1
        LAST = (GL, 2 * GL + 1)

        def tile_geom(g, p, k):
            """-> (d, col0, width, psum offset) for slice k of pair (g,p)."""
            j = 2 * p + k
            d = j - 4 * g
            if 0 <= d <= 3:
                w = QB - 128 * d
                # keep start=True slices in separate PSUM banks
                off = 0 if k == 0 else QB
                return d, 128 * d, w, off
            return None, 0, QB, k * QB

        def emit_scores(g, p):
            qs0 = g * QB
            ps = ps_pair.tile([P, 1024], f32, name=f"ps_{g}_{p}", tag="pair")
            geom = [tile_geom(g, p, k) for k in range(2)]
            for k in range(2):
                j = 2 * p + k
                d, col0, w, off = geom[k]
                nc.tensor.matmul(
                    ps[:, off : off + w],
                    kt_sb[:, j * P : (j + 1) * P],
                    qt_sb[:, qs0 + col0 : qs0 + QB],
                    start=True,
                    stop=True,
                )
            if (g, p) == LAST:
                for k in range(2):
                    d, col0, w, off = geom[k]
                    nc.vector.tensor_add(
                        ps[:, off : off + w],
                        ps[:, off : off + w],
                        mbig[:, 384 : 384 + w],
                    )
            w0 = geom[0][2]
            w1 = geom[1][2]
            expst = expst_pool.tile([P, 1024], bf16, name=f"e{g}_{p}", tag="expst")
            if geom[1][3] == w0:  # contiguous psum: single exp
                nc.scalar.activation(
                    expst[:, : w0 + w1],
                    ps[:, : w0 + w1],
                    mybir.ActivationFunctionType.Exp,
                )
            else:
                nc.scalar.activation(
                    expst[:, :w0], ps[:, :w0], mybir.ActivationFunctionType.Exp
                )
                nc.scalar.activation(
                    expst[:, w0 : w0 + w1],
                    ps[:, QB : QB + w1],
                    mybir.ActivationFunctionType.Exp,
                )
            if (g, p) != LAST:
                for k in range(2):
                    d, col0, w, off = geom[k]
                    if d is not None:
                        e0 = 0 if k == 0 else w0
                        nc.gpsimd.affine_select(
                            out=expst[:, e0 : e0 + w],
                            in_=expst[:, e0 : e0 + w],
                            compare_op=mybir.AluOpType.is_ge,
                            fill=0.0,
                            base=0,
                            pattern=[[1, w]],
                            channel_multiplier=-1,
                        )
            return expst

        def emit_av(g, p, expst):
            npair = 2 * g + 2
            njt = 4 * g + 4
            if p == 0:
                state[g] = [
                    ps_av.tile([P, QB], f32, name=f"pav{g}", tag="pav"),
                    sacc_pool.tile([P, QB], f32, name=f"sacc{g}", tag="sacc"),
                    None,
                ]
            pav, sacc = state[g][0], state[g][1]
            geom = [tile_geom(g, p, k) for k in range(2)]
            w0 = geom[0][2]
            eslice = [expst[:, :w0], expst[:, w0 : w0 + geom[1][2]]]
            for k in range(2):
                j = 2 * p + k
                d, col0, w, off = geom[k]
                nc.tensor.matmul(
                    pav[:, col0:QB],
                    v_sb[:, j * H : (j + 1) * H],
                    eslice[k],
                    start=(j == 0),
                    stop=(j == njt - 1),
                )
            if g == GL and p == npair - 2:
                # row-sums for the last two pairs finish on PE: partial
                # ones-matmul over the DVE-accumulated pairs 0..npair-3, then
                # this pair's two slices
                psum = ps_small.tile([P, QB], f32, name=f"psum{g}", tag="small")
                state[g].append(psum)
                nc.tensor.matmul(
                    psum, ones512[:, :P], state[g][2], start=True, stop=False
                )
                for k in range(2):
                    d, col0, w, off = geom[k]
                    nc.tensor.matmul(
                        psum[:, col0:QB],
                        ones512[:, :P],
                        eslice[k],
                        start=False,
                        stop=False,
                    )
                return
            if (g, p) == LAST:
                psum = state[g][3]
                for k in range(2):
                    d, col0, w, off = geom[k]
                    nc.tensor.matmul(
                        psum[:, col0:QB],
                        ones512[:, :P],
                        eslice[k],
                        start=False,
                        stop=(k == 1),
                    )
                qs0 = g * QB
                for h in range(2):
                    hs = slice(h * 256, (h + 1) * 256)
                    rec = rec_pool.tile([P, QB], f32, name=f"rec{g}{h}", tag="rec")
                    nc.vector.reciprocal_approx_fast(rec[:, :256], psum[:, hs])
                    o = outp.tile([P, QB], f32, name=f"o{g}{h}", tag="o")
                    nc.vector.tensor_mul(o[:, :256], pav[:, hs], rec[:, :256])
                    eng = nc.sync if h == 0 else nc.scalar
                    eng.dma_start(
                        outT[:, qs0 + h * 256 : qs0 + (h + 1) * 256], o[:, :256]
                    )
                return
            # row-sum accumulation: GpSimd folds, DVE accumulates
            if geom[0][0] is None and geom[1][0] is None:
                if p == 0:
                    nc.vector.tensor_add(sacc, eslice[0], eslice[1])
                else:
                    tp = tp_pool.tile([P, QB], bf16, name=f"tp{g}_{p}", tag="tp")
                    nc.vector.tensor_add(tp, eslice[0], eslice[1])
                    nc.vector.tensor_add(sacc, sacc, tp)
            else:
                for k in range(2):
                    d, col0, w, off = geom[k]
                    if p == 0 and k == 0:
                        nc.vector.tensor_copy(sacc[:, col0:QB], eslice[k])
                    else:
                        nc.vector.tensor_add(
                            sacc[:, col0:QB], sacc[:, col0:QB], eslice[k]
                        )
            if (g != GL and p == npair - 1) or (g == GL and p == npair - 3):
                sbf = sbf_pool.tile([P, QB], bf16, name=f"sbf{g}", tag="sbf")
                nc.vector.tensor_copy(sbf, sacc)
                state[g][2] = sbf

        def emit_ep(g):
            qs0 = g * QB
            psum = ps_small.tile([P, QB], f32, name=f"psum{g}", tag="small")
            nc.tensor.matmul(psum, ones512[:, :P], state[g][2], start=True, stop=True)
            rec = rec_pool.tile([P, QB], f32, name=f"rec{g}", tag="rec")
            nc.vector.reciprocal_approx_fast(rec, psum)
            o = outp.tile([P, QB], f32, name=f"o{g}", tag="o")
            nc.vector.tensor_mul(o, state[g][0], rec)
            nc.sync.dma_start(outT[:, qs0 : qs0 + QB], o)

        # --- emission order: proj chunks 2,3 split per-projection and
        # slotted between attention pairs so the PE hides exp latency ---
        S, AV, PC, P1, EP = "S", "AV", "P", "p", "E"
        order = [
            (PC, 0), (PC, 1), (S, 0, 0), (S, 0, 1), (P1, 2, "wq"), (AV, 0, 0),
            (P1, 2, "wk"), (AV, 0, 1), (P1, 2, "wv"), (S, 1, 0), (S, 1, 1),
            (P1, 3, "wq"), (AV, 1, 0), (P1, 3, "wk"), (EP, 0), (AV, 1, 1),
            (S, 1, 2), (P1, 3, "wv"), (S, 1, 3), (AV, 1, 2), (S, 2, 0),
            (AV, 1, 3), (S, 2, 1), (EP, 1), (AV, 2, 0), (S, 2, 2), (AV, 2, 1),
            (S, 2, 3), (AV, 2, 2), (S, 2, 4), (AV, 2, 3), (S, 2, 5),
            (AV, 2, 4), (S, 3, 0), (AV, 2, 5), (S, 3, 1), (EP, 2), (AV, 3, 0),
            (S, 3, 2), (AV, 3, 1), (S, 3, 3), (AV, 3, 2), (S, 3, 4),
            (AV, 3, 3), (S, 3, 5), (AV, 3, 4), (S, 3, 6), (AV, 3, 5),
            (S, 3, 7), (AV, 3, 6), (AV, 3, 7),
        ]

        exps = {}
        for u in order:
            if u[0] == PC:
                emit_proj_chunk(u[1], pairqk=True)
            elif u[0] == P1:
                emit_proj_one(u[1], u[2])
            elif u[0] == S:
                exps[(u[1], u[2])] = emit_scores(u[1], u[2])
            elif u[0] == EP:
                emit_ep(u[1])
            else:
                emit_av(u[1], u[2], exps.pop((u[1], u[2])))

    nc.compile()
    return nc


def _get_bass():
    if "k" not in _CACHE:
        _CACHE["k"] = _build()
    return _CACHE["k"]


LAST_RESULT = None  # BassKernelResults of the most recent kernel() call


def kernel(x, Wq, Wk, Wv):
    global LAST_RESULT
    from concourse.bass_utils import run_bass_kernel_spmd

    np_dt = ml_dtypes.bfloat16

    def _wlayout(w):  # [C, H] -> [P, NCC*H]: sbuf layout, contiguous DMA
        w = np.asarray(w, np.float32).reshape(NCC, P, H).transpose(1, 0, 2)
        return np.ascontiguousarray(w.reshape(P, NCC * H)).astype(np_dt)

    wq_s = _wlayout(np.asarray(Wq, np.float32) * SCALE)
    wk_s = _wlayout(Wk)
    wv_s = _wlayout(Wv)
    x = np.asarray(x, np.float32)

    in_maps = []
    for b in range(N_CORES):
        in_maps.append(
            {
                "xT": np.ascontiguousarray(x[b].T).astype(np_dt),
                "wq": wq_s,
                "wk": wk_s,
                "wv": wv_s,
            }
        )

    nc = _get_bass()
    res = run_bass_kernel_spmd(nc, in_maps, core_ids=list(range(N_CORES)))
    LAST_RESULT = res
    return np.stack(
        [np.ascontiguousarray(r["outT"].T) for r in res.results], axis=0
    )
